# revision 12
# baseline (speedup 1.0000x reference)
"""DeepseekV3 MoE kernel for 8x TRN2 NeuronCores.

Math: with N_ROUTED == NUM_LOCAL == 8, every top-k index is < NUM_LOCAL, so
the per-token combined routed weight is softmax(top2).sum() == 1.  The whole
module therefore reduces to

    y = down_sh(swiglu_sh(x)) + down_r(swiglu_r(x))

i.e. ONE SwiGLU MLP with concatenated intermediate dim 8192 + 1024 = 9216.

Sharding: tensor-parallel over the concatenated intermediate dim (1152 rows
per core).  Each core reads all 8192 tokens, computes a partial down-proj
output; host sums the 8 partials.

V2 design (vs the fp32r baseline):
  - fp16 matmul operands everywhere (same 1 cycle/row PE rate as fp32r,
    half the DMA bytes and SBUF footprint; fp32 PSUM accumulation).
  - All weights resident in SBUF for the whole kernel (108 KB/partition),
    loaded once per rep instead of re-DMAed every sweep.
  - Token tiles double-buffered and prefetched before phase B of the
    previous sweep so the PE never waits on the x stream.
  - Input DMAs on the sync queue, output DMAs on the vector queue so the
    y writeback cannot head-of-line block the x prefetch.

Device kernel (per core), TB=512 tokens per sweep, 16 sweeps:
  phase A: g/u = Wg/Wu-slice @ x -> a = silu(g) * u   (a: [1152, TB] fp16)
  phase B: y[TB, 2048] partial = a.T @ Wd-slice        (psum-accum over i)
"""

import os
import sys

import numpy as np

for _p in ("/opt/trn_rl_repo", "/root/.axon_site/_ro/trn_rl_repo"):
    if os.path.isdir(_p):
        if _p not in sys.path:
            sys.path.insert(0, _p)
        break

from concourse import bacc, mybir, tile  # noqa: E402
from concourse.bass_utils import run_bass_kernel_spmd  # noqa: E402
import ml_dtypes  # noqa: E402

NPF16 = ml_dtypes.bfloat16

N_CORES = 8
H = 2048          # hidden
I_TOT = 9216      # 8192 shared + 1024 routed intermediate
T = 8192          # tokens (4 * 2048)
IC = I_TOT // N_CORES   # 1152 intermediate rows per core
TB = 512          # tokens per sweep
NS = T // TB      # 16 sweeps
KT = H // 128     # 16 contraction tiles for phase A
NI = IC // 128    # 9 intermediate 128-blocks per core
HBN = H // 512    # 4 output column blocks (phase B moving dim)
TSN = TB // 128   # 4 token 128-blocks per sweep (phase B output partition)

F32 = mybir.dt.float32
F16 = mybir.dt.bfloat16  # operand dtype (bf16: better chain rate than f16 on hw)
SILU = mybir.ActivationFunctionType.Silu


def build_nc(n_cores=N_CORES, reps=1):
    nc = bacc.Bacc("TRN2", target_bir_lowering=False, debug=False,
                   num_devices=n_cores)
    xt_d = nc.declare_dram_parameter("xt", [NS, 128, KT, TB], F16, isOutput=False)
    wg_d = nc.declare_dram_parameter("wg", [NI, 128, KT, 128], F16, isOutput=False)
    wu_d = nc.declare_dram_parameter("wu", [NI, 128, KT, 128], F16, isOutput=False)
    wd_d = nc.declare_dram_parameter("wd", [HBN, 128, NI, 512], F16, isOutput=False)
    y_d = nc.declare_dram_parameter("y", [HBN, T, 512], F16, isOutput=True)

    with tile.TileContext(nc) as tc:
        import contextlib
        with (
            tc.tile_pool(name="xp", bufs=2) as xp,
            tc.tile_pool(name="wgp", bufs=NI) as wgp,
            tc.tile_pool(name="wup", bufs=NI) as wup,
            tc.tile_pool(name="wdp", bufs=HBN) as wdp,
            tc.tile_pool(name="apool", bufs=NI) as apool,
            tc.tile_pool(name="actp", bufs=4) as actp,
            tc.tile_pool(name="yp", bufs=6) as ypool,
            tc.tile_pool(name="psA", bufs=4, space="PSUM") as psA,
            tc.tile_pool(name="psY", bufs=4, space="PSUM") as psY,
            tc.For_i(0, reps, 1, staggered_reset=True)
            if reps > 1 else contextlib.nullcontext(),
        ):
            # Token tile for sweep 0 first: the very first matmul needs it.
            xts = [None] * NS
            xts[0] = xp.tile([128, KT, TB], F16, tag="xt", name="xt0")
            nc.sync.dma_start(xts[0][:], xt_d[0])

            # Weights, interleaved in first-use order so phase A of sweep 0
            # can start as soon as block 0 lands.
            wg_t, wu_t, wd_t = [], [], []
            for i in range(NI):
                wgt = wgp.tile([128, KT, 128], F16, tag="wg")
                nc.sync.dma_start(wgt[:], wg_d[i])
                wg_t.append(wgt)
                wut = wup.tile([128, KT, 128], F16, tag="wu")
                nc.sync.dma_start(wut[:], wu_d[i])
                wu_t.append(wut)
            for hb in range(HBN):
                wdt = wdp.tile([128, NI, 512], F16, tag="wd")
                nc.sync.dma_start(wdt[:], wd_d[hb])
                wd_t.append(wdt)

            for s in range(NS):
                xt = xts[s]
                a_tiles = []
                for i in range(NI):
                    gp = psA.tile([128, TB], F32, tag="gu")
                    up = psA.tile([128, TB], F32, tag="gu")
                    for k in range(KT):
                        nc.tensor.matmul(gp[:], wg_t[i][:, k, :], xt[:, k, :],
                                         start=(k == 0), stop=(k == KT - 1))
                    for k in range(KT):
                        nc.tensor.matmul(up[:], wu_t[i][:, k, :], xt[:, k, :],
                                         start=(k == 0), stop=(k == KT - 1))
                    sl = actp.tile([128, TB], F32, tag="sl")
                    nc.scalar.activation(sl[:], gp[:], SILU)
                    a_t = apool.tile([128, TB], F16, tag="a")
                    nc.vector.tensor_mul(a_t[:], sl[:], up[:])
                    a_tiles.append(a_t)

                # Prefetch next sweep's tokens ahead of the y writeback.
                if s + 1 < NS:
                    xts[s + 1] = xp.tile([128, KT, TB], F16, tag="xt",
                                         name=f"xt{s + 1}")
                    nc.sync.dma_start(xts[s + 1][:], xt_d[s + 1])

                for hb in range(HBN):
                    for ts in range(TSN):
                        yps = psY.tile([128, 512], F32, tag="y")
                        for i in range(NI):
                            nc.tensor.matmul(
                                yps[:],
                                a_tiles[i][:, ts * 128:(ts + 1) * 128],
                                wd_t[hb][:, i, :],
                                start=(i == 0), stop=(i == NI - 1))
                        ysb = ypool.tile([128, 512], F16, tag="ysb")
                        nc.vector.tensor_copy(ysb[:], yps[:])
                        nc.scalar.dma_start(
                            y_d[hb, s * TB + ts * 128: s * TB + (ts + 1) * 128, :],
                            ysb[:])
    nc.compile()
    return nc


def prep_core_inputs(hidden_states, sh_gate, sh_up, sh_down, r_gate, r_up, r_down):
    """Host-side shard + retile to fp16.  Returns in_maps for
    run_bass_kernel_spmd."""
    x = np.ascontiguousarray(hidden_states, dtype=np.float32).reshape(T, H)
    # xt[s, p, k, t] = x[s*TB + t, k*128 + p]
    xt = np.ascontiguousarray(
        x.reshape(NS, TB, KT, 128).transpose(0, 3, 2, 1).astype(NPF16))

    wg_cat = np.concatenate([sh_gate, r_gate], axis=0)    # [I_TOT, H]
    wu_cat = np.concatenate([sh_up, r_up], axis=0)        # [I_TOT, H]
    wd_cat = np.concatenate([sh_down, r_down], axis=1)    # [H, I_TOT]

    in_maps = []
    for c in range(N_CORES):
        isl = slice(c * IC, (c + 1) * IC)
        # wg_t[b, p, k, m] = wg_cat[c*IC + b*128 + m, k*128 + p]
        wg_t = np.ascontiguousarray(
            wg_cat[isl].reshape(NI, 128, KT, 128)
            .transpose(0, 3, 2, 1).astype(NPF16))
        wu_t = np.ascontiguousarray(
            wu_cat[isl].reshape(NI, 128, KT, 128)
            .transpose(0, 3, 2, 1).astype(NPF16))
        # wd_t[hb, p, b, hcol] = wd_cat[hb*512 + hcol, c*IC + b*128 + p]
        wdslice = np.ascontiguousarray(wd_cat[:, isl].T)  # [IC, H]
        wd_t = np.ascontiguousarray(
            wdslice.reshape(NI, 128, HBN, 512)
            .transpose(2, 1, 0, 3).astype(NPF16))
        in_maps.append({"xt": xt, "wg": wg_t, "wu": wu_t, "wd": wd_t})
    return in_maps


_NC_CACHE = {}


def _get_nc():
    if "nc" not in _NC_CACHE:
        _NC_CACHE["nc"] = build_nc()
    return _NC_CACHE["nc"]


def run(in_maps, trace=False, **kw):
    nc = _get_nc()
    return run_bass_kernel_spmd(nc, in_maps, list(range(N_CORES)),
                                trace=trace, **kw)


class Runner:
    """Persistent sharded-jit executor: stage inputs to device once, then
    time repeated kernel executions without re-trace/transfer overhead."""

    def __init__(self, nc=None):
        import jax
        import mybir
        from jax.sharding import Mesh, PartitionSpec
        from jax.experimental.shard_map import shard_map
        from concourse import bass2jax

        self._jax = jax
        if nc is None:
            nc = _get_nc()
        bass2jax.install_neuronx_cc_hook()

        partition_name = (nc.partition_id_tensor.name
                          if nc.partition_id_tensor else None)
        in_names, out_names, out_avals = [], [], []
        for alloc in nc.m.functions[0].allocations:
            if not isinstance(alloc, mybir.MemoryLocationSet):
                continue
            name = alloc.memorylocations[0].name
            if alloc.kind == "ExternalInput":
                if name != partition_name:
                    in_names.append(name)
            elif alloc.kind == "ExternalOutput":
                out_names.append(name)
                out_avals.append(jax.core.ShapedArray(
                    tuple(alloc.tensor_shape), mybir.dt.np(alloc.dtype)))
        self._in_names, self._out_names, self._out_avals = \
            in_names, out_names, out_avals
        all_in = in_names + out_names
        if partition_name is not None:
            all_in = all_in + [partition_name]

        def _body(*args):
            operands = list(args)
            if partition_name is not None:
                operands.append(bass2jax.partition_id_tensor())
            outs = bass2jax._bass_exec_p.bind(
                *operands,
                out_avals=tuple(out_avals),
                in_names=tuple(all_in),
                out_names=tuple(out_names),
                lowering_input_output_aliases=(),
                sim_require_finite=True,
                sim_require_nnan=True,
                nc=nc,
            )
            return tuple(outs)

        self._body = _body
        self._chain_cache = {}
        devices = jax.devices()[:N_CORES]
        self._mesh = Mesh(np.asarray(devices), ("core",))
        n_all = len(in_names) + len(out_names)
        self._sharded = jax.jit(shard_map(
            _body, mesh=self._mesh,
            in_specs=(PartitionSpec("core"),) * n_all,
            out_specs=(PartitionSpec("core"),) * len(out_names),
            check_rep=False))
        self._dev_args = None

    def stage(self, in_maps):
        import jax
        from jax.sharding import NamedSharding, PartitionSpec

        sh = NamedSharding(self._mesh, PartitionSpec("core"))
        args = []
        for name in self._in_names:
            cat = np.concatenate([np.asarray(m[name]) for m in in_maps], axis=0)
            args.append(jax.device_put(cat, sh))
        for av in self._out_avals:
            z = np.zeros((N_CORES * av.shape[0], *av.shape[1:]), av.dtype)
            args.append(jax.device_put(z, sh))
        jax.block_until_ready(args)
        self._dev_args = args

    def execute(self):
        out = self._sharded(*self._dev_args)
        self._jax.block_until_ready(out)
        return out

    def execute_chain(self, k):
        """Run the kernel k times back-to-back inside one jit dispatch.
        Successive calls chain the output buffers, so device executions
        serialize; wall-time differences measure pure device time."""
        import jax
        from jax.experimental.shard_map import shard_map
        from jax.sharding import PartitionSpec

        if k not in self._chain_cache:
            n_in = len(self._in_names)
            n_out = len(self._out_names)
            body = self._body

            def _chain(*args):
                ins = args[:n_in]
                outs = args[n_in:]
                for _ in range(k):
                    outs = body(*ins, *outs)
                return outs

            self._chain_cache[k] = jax.jit(shard_map(
                _chain, mesh=self._mesh,
                in_specs=(PartitionSpec("core"),) * (n_in + n_out),
                out_specs=(PartitionSpec("core"),) * n_out,
                check_rep=False))
        out = self._chain_cache[k](*self._dev_args)
        self._jax.block_until_ready(out)
        return out

    def results(self, out):
        per_core = []
        for c in range(N_CORES):
            d = {}
            for i, name in enumerate(self._out_names):
                av = self._out_avals[i]
                d[name] = np.asarray(out[i]).reshape(N_CORES, *av.shape)[c]
            per_core.append(d)
        return per_core


def kernel(hidden_states, router_weight, sh_gate, sh_up, sh_down,
           r_gate, r_up, r_down):
    in_maps = prep_core_inputs(hidden_states, sh_gate, sh_up, sh_down,
                               r_gate, r_up, r_down)
    res = run(in_maps)
    acc = np.zeros((HBN, T, 512), np.float64)
    for c in range(N_CORES):
        acc += res.results[c]["y"].astype(np.float64)
    out = acc.transpose(1, 0, 2).reshape(T, H)
    return np.ascontiguousarray(out).astype(np.float32).reshape(
        hidden_states.shape)
